# revision 6
# baseline (speedup 1.0000x reference)
"""Associative-embedding loss (push/pull) on 8 TRN2 NeuronCores.

Strategy (pure data parallel, hardcoded):
  - B=32 images, 8 cores -> 4 images per core.
  - Per image only 510 of the 278528 tag rows are needed, so the kernel
    never streams the tags tensor. The rows are fetched with FOUR
    multi-descriptor `dma_gather` instructions (one per image, 512
    descriptors each) instead of 16 single-column SWDGE indirect DMAs:
    the SWDGE cost is ~994 ns fixed + 0.34 ns/descriptor, so 4x512
    costs ~4.7 us of GpSimd time where 16x128 costs ~17.6 us.
    dma_gather requires 256B-aligned elements and int16 indices, so
    each descriptor fetches the 256B chunk (16 tag rows) containing the
    keypoint's row (chunk index = row//16 < 17408 fits int16 per
    image); the wanted 16B row is then extracted with a host-built
    one-hot mask multiply + strided reduce on DVE, pipelined under the
    later gathers.
  - All per-image compute is image-parallel (engine APs must start at
    partition 0 / quadrant boundaries, so the layout keeps every
    operand base at partition 0):
      * slot-matmuls: 4 accumulating matmuls (one per slot column j)
        with lhsT = gathered rows [128, 16(b,d)] and rhs = per-slot
        membership-with-visibility [128, 120(b,person)] producing
        S[16(b,d), 120(b,person)] in PSUM.
      * means: S * block-diagonal 1/cnt map -> meanf [16, 120]; the
        map's zeros kill the cross-image PSUM blocks, so later
        16-partition contractions see only the owning image's rows.
      * q duplicated into two PSUM rows via a [16,2] ones matmul, then
        massaged into A=[q;1], B=[1;q] rows with per-partition-scalar
        selects; diff[i,j] = q_i + q_j - 2<mi,mj> needs just TWO
        matmuls per image: n2mf^T @ meanf (16-part contraction over the
        block-diagonal means) accumulated with A^T @ B.
      * exp / (!=0)*hmask mask / row-reduce / final scalings operate on
        the full [30, 120] block at once.
  - Everything that depends only on `keypoints` (visibility, counts,
    pair masks, extraction masks, scales) is precomputed on the host.

Inputs: tags [32, 278528, 4] f32, keypoints [32, 30, 17, 2] int.
Output: [32, 2] f32 (push, pull) per image.
"""

import numpy as np

import concourse.bacc as bacc
import concourse.bass as bass
import concourse.mybir as mybir
import concourse.tile as tile
from concourse.bass_utils import run_bass_kernel_spmd

B, N, D = 32, 278528, 4
NPERS, NKP = 30, 17
NFLAT = NPERS * NKP          # 510 keypoints per image
KPJ = 4                      # keypoint slot-columns (4 x 128 = 512 slots)
NCORES = 8
IMGS = B // NCORES           # 4 images per core
W = IMGS * NPERS             # 120: merged free width
BD = IMGS * D                # 16: merged (image, dim) rows
CHW = 16                     # tag rows per 256B gather chunk
ELEM = CHW * D               # 64 floats per gather chunk
NCH = N // CHW               # 17408 chunks per image (fits int16)
NI = 128 * KPJ               # 512 descriptors per image
EPS = 1e-6
X = mybir.AxisListType


def _build_nc():
    nc = bacc.Bacc("TRN2", target_bir_lowering=False, debug=False)
    f32 = mybir.dt.float32
    tags = nc.dram_tensor("tags", [IMGS * N, D], f32, kind="ExternalInput").ap()
    idx16 = nc.dram_tensor(
        "idx16", [128, IMGS * (NI // 16)], mybir.dt.int16, kind="ExternalInput"
    ).ap()
    emask = nc.dram_tensor("emask", [128, IMGS * KPJ * ELEM], f32, kind="ExternalInput").ap()
    # big per-slot map: cols j*W + b*30 + person = vis (membership),
    # cols 480 + j*16 + b*4 + d = vis * invcnt (w2 for the pull term 1)
    mvw = nc.dram_tensor("mvw", [128, KPJ * W + BD * KPJ], f32, kind="ExternalInput").ap()
    # small map: rows 0:30 cols 0:120 hmask; rows 0:16 cols 120:240
    # block-diagonal 1/cnt; row 0 cols 240:248 scales (push x4, pull x4)
    cst2 = nc.dram_tensor("cst2", [30, 248], f32, kind="ExternalInput").ap()
    out = nc.dram_tensor("out", [1, IMGS * 2], f32, kind="ExternalOutput").ap()

    tags_ch = tags.rearrange("(a c) d -> a (c d)", c=CHW)   # [IMGS*NCH, 64]

    with tile.TileContext(nc) as tc:
        with (
            tc.tile_pool(name="const", bufs=1) as cpool,
            tc.tile_pool(name="psum", bufs=1, space="PSUM") as ppool,
        ):
            # ---- critical path: index load, then one dma_gather per image
            idx_t = cpool.tile([128, IMGS * (NI // 16)], mybir.dt.int16)
            nc.scalar.dma_start(idx_t[:], idx16)
            ch_t = cpool.tile([128, IMGS * KPJ * ELEM], f32)
            for b in range(IMGS):
                nc.gpsimd.dma_gather(
                    ch_t[:, b * KPJ * ELEM:(b + 1) * KPJ * ELEM].rearrange(
                        "p (k f) -> p k f", f=ELEM
                    ),
                    tags_ch[b * NCH:(b + 1) * NCH, :],
                    idx_t[:, b * (NI // 16):(b + 1) * (NI // 16)],
                    NI, NI, ELEM,
                )

            # ---- constants (overlap the gathers)
            emask_t = cpool.tile([128, IMGS * KPJ * ELEM], f32)
            nc.sync.dma_start(emask_t[:], emask)
            mvw_t = cpool.tile([128, KPJ * W + BD * KPJ], f32)
            nc.sync.dma_start(mvw_t[:], mvw)
            cst2_t = cpool.tile([30, 248], f32)
            nc.scalar.dma_start(cst2_t[:], cst2)
            hmask = cst2_t[0:30, 0:W]
            inv4full = cst2_t[0:BD, W:2 * W]
            scales = cst2_t[0:1, 2 * W:2 * W + 2 * IMGS]

            ones16_2t = cpool.tile([BD, 2], f32)
            nc.vector.memset(ones16_2t[:], 1.0)
            ones30_t = cpool.tile([NPERS, 1], f32)
            nc.vector.memset(ones30_t[:], 1.0)
            ones128_t = cpool.tile([128, 1], f32)
            nc.vector.memset(ones128_t[:], 1.0)
            # row-select masks to build A=[q;1], B=[1;q] from the
            # duplicated-q PSUM rows without partition-offset writes
            sel_a = cpool.tile([2, 1], f32)
            nc.vector.memset(sel_a[:], 0.0)
            nc.vector.memset(sel_a[0:1, :], 1.0)
            sel_b = cpool.tile([2, 1], f32)
            nc.vector.memset(sel_b[:], 1.0)
            nc.vector.memset(sel_b[0:1, :], 0.0)
            inv_a = cpool.tile([2, W], f32)
            nc.vector.memset(inv_a[:], 1.0)
            nc.vector.memset(inv_a[0:1, :], 0.0)
            inv_b = cpool.tile([2, W], f32)
            nc.vector.memset(inv_b[:], 0.0)
            nc.vector.memset(inv_b[0:1, :], 1.0)

            # ---- extract the wanted 16B row from each 256B chunk
            # (per image, pipelined under the later gathers)
            g_t = cpool.tile([128, KPJ * BD], f32)
            mm_t = cpool.tile([128, IMGS * KPJ * ELEM], f32)
            g4 = g_t[:].rearrange("p (j b2 d) -> p j b2 d", j=KPJ, b2=IMGS)
            for b in range(IMGS):
                csl = slice(b * KPJ * ELEM, (b + 1) * KPJ * ELEM)
                nc.vector.tensor_mul(mm_t[:, csl], ch_t[:, csl], emask_t[:, csl])
                nc.vector.reduce_sum(
                    g4[:, :, b:b + 1, :],
                    mm_t[:, csl].rearrange("p (j r d) -> p j d r", j=KPJ, r=CHW),
                    axis=X.X,
                )

            # ---- pull term 1 pieces on DVE (parallel with PE chain)
            u_t = cpool.tile([128, KPJ * BD], f32)
            nc.vector.tensor_mul(u_t[:], g_t[:], mvw_t[:, KPJ * W:])
            u2_t = cpool.tile([128, KPJ * BD], f32)
            nc.vector.tensor_mul(u2_t[:], u_t[:], g_t[:])
            cols4_t = cpool.tile([128, IMGS], f32)
            nc.vector.reduce_sum(
                cols4_t[:],
                u2_t[:].rearrange("p (j b d) -> p b j d", j=KPJ, b=IMGS),
                axis=X.XY,
            )

            # ---- per-slot accumulating matmuls -> S[(b,d), (b,person)]
            s_p = ppool.tile([BD, W], f32, space="PSUM")
            for j in range(KPJ):
                nc.tensor.matmul(
                    out=s_p[:],
                    lhsT=g_t[:, j * BD:(j + 1) * BD],
                    rhs=mvw_t[:, j * W:(j + 1) * W],
                    start=(j == 0),
                    stop=(j == KPJ - 1),
                )

            # means (block-diagonal map zeroes cross-image sums)
            meanf_t = cpool.tile([BD, W], f32)
            nc.vector.tensor_mul(meanf_t[:], s_p[:], inv4full)
            n2mf_t = cpool.tile([BD, W], f32)
            nc.vector.tensor_scalar_mul(n2mf_t[:], meanf_t[:], -2.0)
            sqm_t = cpool.tile([BD, W], f32)
            nc.scalar.square(sqm_t[:], meanf_t[:])
            q2_p = ppool.tile([2, W], f32, space="PSUM")
            nc.tensor.matmul(
                out=q2_p[:], lhsT=ones16_2t[:], rhs=sqm_t[:], start=True, stop=True
            )
            # A = [q; ones], B = [ones; q]
            a_t = cpool.tile([2, W], f32)
            nc.vector.scalar_tensor_tensor(
                a_t[:], q2_p[:], sel_a[:], inv_a[:],
                op0=mybir.AluOpType.mult, op1=mybir.AluOpType.add,
            )
            b_t = cpool.tile([2, W], f32)
            nc.vector.scalar_tensor_tensor(
                b_t[:], q2_p[:], sel_b[:], inv_b[:],
                op0=mybir.AluOpType.mult, op1=mybir.AluOpType.add,
            )

            # diff[i, (b,j)] = q_i + q_j - 2<mi,mj>, two matmuls per image
            dall_p = ppool.tile([NPERS, W], f32, space="PSUM")
            for b in range(IMGS):
                c30 = slice(b * NPERS, (b + 1) * NPERS)
                nc.tensor.matmul(
                    out=dall_p[:, c30], lhsT=n2mf_t[:, c30], rhs=meanf_t[:, c30],
                    start=True, stop=False,
                )
                nc.tensor.matmul(
                    out=dall_p[:, c30], lhsT=a_t[:, c30], rhs=b_t[:, c30],
                    start=False, stop=True,
                )

            # push: exp(-diff) * (diff != 0) * hmask, reduced per image
            e_t = cpool.tile([NPERS, W], f32)
            nc.scalar.activation(
                e_t[:], dall_p[:],
                mybir.ActivationFunctionType.Exp, bias=0.0, scale=-1.0,
            )
            m2_t = cpool.tile([NPERS, W], f32)
            nc.vector.scalar_tensor_tensor(
                m2_t[:], dall_p[:], 0.0, hmask,
                op0=mybir.AluOpType.not_equal, op1=mybir.AluOpType.mult,
            )
            c_t = cpool.tile([NPERS, W], f32)
            nc.vector.tensor_mul(c_t[:], e_t[:], m2_t[:])
            prow_t = cpool.tile([NPERS, IMGS], f32)
            nc.vector.reduce_sum(
                prow_t[:],
                c_t[:].rearrange("p (i q) -> p i q", q=NPERS),
                axis=X.X,
            )
            pt_p = ppool.tile([1, IMGS], f32, space="PSUM")
            nc.tensor.matmul(
                out=pt_p[:], lhsT=ones30_t[:], rhs=prow_t[:], start=True, stop=True
            )

            # pull: term1 (gathered-row dot) - term2 (sum of q)
            t1_p = ppool.tile([1, IMGS], f32, space="PSUM")
            nc.tensor.matmul(
                out=t1_p[:], lhsT=ones128_t[:], rhs=cols4_t[:], start=True, stop=True
            )
            term2_t = cpool.tile([1, IMGS], f32)
            nc.vector.reduce_sum(
                term2_t[:],
                q2_p[0:1, :].rearrange("o (i p) -> o i p", p=NPERS),
                axis=X.X,
            )
            pull4_t = cpool.tile([1, IMGS], f32)
            nc.vector.tensor_sub(pull4_t[:], t1_p[:], term2_t[:])

            res_t = cpool.tile([1, IMGS * 2], f32)
            r3 = res_t[:].rearrange("o (i t) -> o i t", t=2)
            nc.vector.tensor_mul(
                r3[:, :, 0:1],
                pt_p[:].rearrange("o (i u) -> o i u", u=1),
                scales[:, 0:IMGS].rearrange("o (i u) -> o i u", u=1),
            )
            nc.vector.tensor_mul(
                r3[:, :, 1:2],
                pull4_t[:].rearrange("o (i u) -> o i u", u=1),
                scales[:, IMGS:2 * IMGS].rearrange("o (i u) -> o i u", u=1),
            )
            nc.sync.dma_start(out, res_t[:])

    nc.compile()
    return nc


_NC_CACHE = None


def _get_nc():
    global _NC_CACHE
    if _NC_CACHE is None:
        _NC_CACHE = _build_nc()
    return _NC_CACHE


def _host_prep(tags: np.ndarray, keypoints: np.ndarray):
    """Build the per-core input maps. tags [B,N,D] f32, keypoints [B,30,17,2].

    Keypoint m (0..509) of an image maps to descriptor m of that image's
    dma_gather: SBUF partition m%128, slot-column j=m//128. person(m)=m//17.
    """
    kp_idx = keypoints[..., 0].reshape(B, NFLAT).astype(np.int64)
    kp_vis = (keypoints[..., 1] > 0).reshape(B, NFLAT)
    upper = np.triu(np.ones((NPERS, NPERS), dtype=bool), 1)
    m_all = np.arange(NI)
    m_part = m_all % 128                  # partition of keypoint m
    m_col = m_all // 128                  # slot-column j of keypoint m
    m_person = np.minimum(m_all // NKP, NPERS - 1)
    m_valid = m_all < NFLAT

    in_maps = []
    for c in range(NCORES):
        tags_flat = np.ascontiguousarray(
            tags[c * IMGS:(c + 1) * IMGS].reshape(IMGS * N, D), dtype=np.float32
        )
        idx16 = np.zeros((128, IMGS * (NI // 16)), dtype=np.int16)
        emask = np.zeros((128, IMGS * KPJ * ELEM), dtype=np.float32)
        mvw = np.zeros((128, KPJ * W + BD * KPJ), dtype=np.float32)
        cst2 = np.zeros((30, 248), dtype=np.float32)
        for lb in range(IMGS):
            gb = c * IMGS + lb
            fidx = kp_idx[gb]            # [510] tag-row index per keypoint
            fvis = kp_vis[gb]            # [510]
            rows = np.zeros(NI, dtype=np.int64)
            rows[:NFLAT] = fidx
            vis = np.zeros(NI, dtype=np.float32)
            vis[:NFLAT] = fvis.astype(np.float32)
            # gather indices: descriptor m -> chunk rows[m]//16, wrapped
            # [m%16, m//16] within this image's 32-column block
            blk = np.zeros((16, NI // 16), dtype=np.int16)
            blk[m_all % 16, m_all // 16] = (rows // CHW).astype(np.int16)
            idx16[:, lb * (NI // 16):(lb + 1) * (NI // 16)] = np.tile(blk, (8, 1))
            # extraction one-hot: [p, j*ELEM + (row%16)*4 + d] = 1
            r_in = rows % CHW
            for d in range(D):
                emask[
                    m_part,
                    lb * KPJ * ELEM + m_col * ELEM + r_in * D + d,
                ] = m_valid
            # membership with visibility folded in
            mv = vis * m_valid
            mvw[m_part, m_col * W + lb * NPERS + m_person] = mv
            # per-person counts / validity
            vis_pk = fvis.reshape(NPERS, NKP)
            cnt = vis_pk.sum(axis=1).astype(np.float32)
            valid = cnt > 0
            safe_cnt = np.maximum(cnt, 1.0)
            invcv = valid / safe_cnt
            # w2 = vis * invcnt, replicated over d, in (j, b, d) layout
            w2v = mv * invcv[m_person]
            for d in range(D):
                mvw[m_part, KPJ * W + m_col * BD + lb * D + d] = w2v
            # block-diagonal 1/cnt map rows (b*4+d)
            cst2[lb * D:(lb + 1) * D, W + lb * NPERS:W + (lb + 1) * NPERS] = (
                1.0 / safe_cnt
            )[None, :]
            cst2[0:NPERS, lb * NPERS:(lb + 1) * NPERS] = (
                upper & valid[:, None] & valid[None, :]
            ).astype(np.float32)
            n = valid.sum().astype(np.float32)
            cst2[0, 2 * W + lb] = 1.0 / ((n - 1.0) * n + EPS)
            cst2[0, 2 * W + IMGS + lb] = 1.0 / (n + EPS)
        in_maps.append(
            {
                "tags": tags_flat,
                "idx16": idx16,
                "emask": emask,
                "mvw": mvw,
                "cst2": cst2,
            }
        )
    return in_maps


def kernel(tags: np.ndarray, keypoints: np.ndarray) -> np.ndarray:
    tags = np.asarray(tags, dtype=np.float32)
    keypoints = np.asarray(keypoints)
    nc = _get_nc()
    in_maps = _host_prep(tags, keypoints)
    res = run_bass_kernel_spmd(nc, in_maps, core_ids=list(range(NCORES)))
    outs = [np.asarray(r["out"]).reshape(IMGS, 2) for r in res.results]
    return np.concatenate(outs, axis=0)


# revision 7
# speedup vs baseline: 1.2327x; 1.2327x over previous
"""Associative-embedding loss (push/pull) on 8 TRN2 NeuronCores.

Strategy (pure data parallel, hardcoded):
  - B=32 images, 8 cores -> 4 images per core.
  - Per image only 510 of the 278528 tag rows are needed, so the kernel
    never streams the tags tensor: 16 indirect (SWDGE) DMAs (one per
    image x slot-column, 128 descriptors each) gather the rows. The
    SWDGE Q7 ucode costs ~1.1 us per instruction nearly independent of
    descriptor count, and only the [128,1]-offset form has correct
    semantics on HW, so 16 instructions (~22 us serialized on GpSimd)
    are the gather floor; everything else hides under that chain.
  - Per-image compute pipelines under the gather chain (image-major
    gather order). Visibility is folded into the per-image membership
    matrices host-side, so the slot matmuls consume gather output
    directly; the per-image chain is:
      msum (4 matmuls) -> meanT / ACT-square / -2*mean (parallel) ->
      q (1 matmul) -> qrow -> diff (3 matmuls) -> exp / mask ->
      fused mask-multiply + row-reduce.
  - Final push/pull scalars via two ones-matmuls + small DVE ops.
  - Everything that depends only on `keypoints` (visibility, counts,
    pair masks, scales) is precomputed on the host and shipped in two
    constant DMAs.

Inputs: tags [32, 278528, 4] f32, keypoints [32, 30, 17, 2] int.
Output: [32, 2] f32 (push, pull) per image.
"""

import numpy as np

import concourse.bacc as bacc
import concourse.bass as bass
import concourse.mybir as mybir
import concourse.tile as tile
from concourse.bass_utils import run_bass_kernel_spmd

B, N, D = 32, 278528, 4
NPERS, NKP = 30, 17
NFLAT = NPERS * NKP          # 510 keypoints per image
KPJ = 4                      # keypoint slots per SBUF partition (128*4=512)
NCORES = 8
IMGS = B // NCORES           # 4 images per core
W = IMGS * NPERS             # 120: merged free width
EPS = 1e-6
X = mybir.AxisListType


def _build_nc():
    nc = bacc.Bacc("TRN2", target_bir_lowering=False, debug=False)
    f32 = mybir.dt.float32
    tags = nc.dram_tensor("tags", [IMGS * N, D], f32, kind="ExternalInput").ap()
    idx = nc.dram_tensor("idx", [128, IMGS * KPJ], mybir.dt.int32, kind="ExternalInput").ap()
    # member3 with vis folded: cols b*120 + j*30 + person; then w2 cols
    # 480 + b*16 + j*4 + d
    mvw = nc.dram_tensor("mvw", [128, IMGS * W + IMGS * KPJ * D], f32, kind="ExternalInput").ap()
    # rows 0:30 cols 0:120 hmask; rows 0:4: cols 120:240 inv4 (1/cnt),
    # cols 240:360 n2inv4 (-2/cnt); row 0: cols 360:480 inv2 (1/cnt^2),
    # cols 480:488 scales (push x4, pull x4)
    cst2 = nc.dram_tensor("cst2", [30, 488], f32, kind="ExternalInput").ap()
    out = nc.dram_tensor("out", [1, IMGS * 2], f32, kind="ExternalOutput").ap()

    with tile.TileContext(nc) as tc:
        with (
            tc.tile_pool(name="const", bufs=1) as cpool,
            tc.tile_pool(name="work", bufs=4) as wpool,
            tc.tile_pool(name="psum", bufs=1, space="PSUM") as ppool,
        ):
            # critical path first: index loads (col 0 alone so the first
            # gather unblocks early), then the 16 gathers image-major
            idx_t = cpool.tile([128, IMGS * KPJ], mybir.dt.int32)
            nc.sync.dma_start(idx_t[:, 0:1], idx[:, 0:1])
            nc.sync.dma_start(idx_t[:, 1:], idx[:, 1:])
            g_ts = []
            for b in range(IMGS):
                g_t = wpool.tile([128, KPJ * D], f32, tag="g")
                g_ts.append(g_t)
                for j in range(KPJ):
                    nc.gpsimd.indirect_dma_start(
                        out=g_t[:, j * D:(j + 1) * D],
                        out_offset=None,
                        in_=tags,
                        in_offset=bass.IndirectOffsetOnAxis(
                            ap=idx_t[:, b * KPJ + j:b * KPJ + j + 1], axis=0
                        ),
                    )

            mvw_t = cpool.tile([128, IMGS * W + IMGS * KPJ * D], f32)
            nc.scalar.dma_start(mvw_t[:], mvw)
            cst2_t = cpool.tile([30, 488], f32)
            nc.scalar.dma_start(cst2_t[:], cst2)
            hmask = cst2_t[0:30, 0:W]
            inv4 = cst2_t[0:D, W:2 * W]
            n2inv4 = cst2_t[0:D, 2 * W:3 * W]
            inv2 = cst2_t[0:1, 3 * W:4 * W]
            scales = cst2_t[0:1, 4 * W:4 * W + 2 * IMGS]

            ones4_t = cpool.tile([D, 1], f32)
            nc.vector.memset(ones4_t[:], 1.0)
            ones30_t = cpool.tile([NPERS, 1], f32)
            nc.vector.memset(ones30_t[:], 1.0)
            ones128_t = cpool.tile([128, 1], f32)
            nc.vector.memset(ones128_t[:], 1.0)
            ones1_t = cpool.tile([1, NPERS], f32)
            nc.vector.memset(ones1_t[:], 1.0)

            cols_t = cpool.tile([128, IMGS], f32)
            meanT_t = cpool.tile([D, W], f32)
            sqm_t = cpool.tile([D, W], f32)
            n2m_t = cpool.tile([D, W], f32)
            qrow_t = cpool.tile([1, W], f32)
            e_t = cpool.tile([NPERS, W], f32)
            m2_t = cpool.tile([NPERS, W], f32)
            c_t = cpool.tile([NPERS, W], f32)
            prow_t = cpool.tile([NPERS, IMGS], f32)
            dall_p = ppool.tile([NPERS, W], f32, space="PSUM")

            # per-image pipeline (images 0..2 hide under the gather chain)
            for b in range(IMGS):
                c30 = slice(b * NPERS, (b + 1) * NPERS)
                g_t = g_ts[b]
                msum_p = ppool.tile([D, NPERS], f32, space="PSUM", tag="msum", bufs=2)
                for j in range(KPJ):
                    nc.tensor.matmul(
                        out=msum_p[:],
                        lhsT=g_t[:, j * D:(j + 1) * D],
                        rhs=mvw_t[:, b * W + j * NPERS:b * W + (j + 1) * NPERS],
                        start=(j == 0),
                        stop=(j == KPJ - 1),
                    )
                # pull term 1: sum over slots of vis*invcnt*|row|^2
                w2sl = slice(
                    IMGS * W + b * KPJ * D, IMGS * W + (b + 1) * KPJ * D
                )
                u_t = wpool.tile([128, KPJ * D], f32, tag="u", bufs=2)
                nc.vector.tensor_mul(u_t[:], g_t[:], mvw_t[:, w2sl])
                u2_t = wpool.tile([128, KPJ * D], f32, tag="u2", bufs=2)
                nc.vector.tensor_mul(u2_t[:], u_t[:], g_t[:])
                nc.vector.reduce_sum(cols_t[:, b:b + 1], u2_t[:], axis=X.X)
                # means and q pieces (ACT square in parallel with DVE muls)
                nc.vector.tensor_mul(meanT_t[:, c30], msum_p[:], inv4[:, c30])
                nc.scalar.square(sqm_t[:, c30], msum_p[:])
                nc.vector.tensor_mul(n2m_t[:, c30], msum_p[:], n2inv4[:, c30])
                q_p = ppool.tile([1, NPERS], f32, space="PSUM", tag="q", bufs=2)
                nc.tensor.matmul(
                    out=q_p[:], lhsT=ones4_t[:], rhs=sqm_t[:, c30],
                    start=True, stop=True,
                )
                nc.vector.tensor_mul(qrow_t[:, c30], q_p[:], inv2[:, c30])
                # diff[i,j] = q_i + q_j - 2<mi,mj>
                nc.tensor.matmul(
                    out=dall_p[:, c30], lhsT=n2m_t[:, c30], rhs=meanT_t[:, c30],
                    start=True, stop=False,
                )
                nc.tensor.matmul(
                    out=dall_p[:, c30], lhsT=qrow_t[:, c30], rhs=ones1_t[:],
                    start=False, stop=False,
                )
                nc.tensor.matmul(
                    out=dall_p[:, c30], lhsT=ones1_t[:], rhs=qrow_t[:, c30],
                    start=False, stop=True,
                )
                # push piece: exp(-diff) * (diff != 0) * hmask, row-reduced
                nc.scalar.activation(
                    e_t[:, c30], dall_p[:, c30],
                    mybir.ActivationFunctionType.Exp, bias=0.0, scale=-1.0,
                )
                nc.vector.scalar_tensor_tensor(
                    m2_t[:, c30], dall_p[:, c30], 0.0, hmask[:, c30],
                    op0=mybir.AluOpType.not_equal, op1=mybir.AluOpType.mult,
                )
                nc.vector.scalar_tensor_tensor(
                    c_t[:, c30], e_t[:, c30], 1.0, m2_t[:, c30],
                    op0=mybir.AluOpType.mult, op1=mybir.AluOpType.mult,
                    accum_out=prow_t[:, b:b + 1],
                )

            # epilogue: push = scale * sum_person prow, pull = scale * (t1 - term2)
            pt_p = ppool.tile([1, IMGS], f32, space="PSUM")
            nc.tensor.matmul(
                out=pt_p[:], lhsT=ones30_t[:], rhs=prow_t[:], start=True, stop=True
            )
            t1_p = ppool.tile([1, IMGS], f32, space="PSUM")
            nc.tensor.matmul(
                out=t1_p[:], lhsT=ones128_t[:], rhs=cols_t[:], start=True, stop=True
            )
            term2_t = cpool.tile([1, IMGS], f32)
            nc.vector.reduce_sum(
                term2_t[:], qrow_t[:].rearrange("o (i p) -> o i p", p=NPERS),
                axis=X.X,
            )
            pull4_t = cpool.tile([1, IMGS], f32)
            nc.vector.tensor_sub(pull4_t[:], t1_p[:], term2_t[:])
            res_t = cpool.tile([1, IMGS * 2], f32)
            r3 = res_t[:].rearrange("o (i t) -> o i t", t=2)
            nc.vector.tensor_mul(
                r3[:, :, 0:1],
                pt_p[:].rearrange("o (i u) -> o i u", u=1),
                scales[:, 0:IMGS].rearrange("o (i u) -> o i u", u=1),
            )
            nc.vector.tensor_mul(
                r3[:, :, 1:2],
                pull4_t[:].rearrange("o (i u) -> o i u", u=1),
                scales[:, IMGS:2 * IMGS].rearrange("o (i u) -> o i u", u=1),
            )
            nc.sync.dma_start(out, res_t[:])

    nc.compile()
    return nc


_NC_CACHE = None


def _get_nc():
    global _NC_CACHE
    if _NC_CACHE is None:
        _NC_CACHE = _build_nc()
    return _NC_CACHE


def _host_prep(tags: np.ndarray, keypoints: np.ndarray):
    """Build the per-core input maps. tags [B,N,D] f32, keypoints [B,30,17,2].

    Slot s = p*KPJ + j (partition p, slot-column j); person(s) = s//17.
    """
    kp_idx = keypoints[..., 0].reshape(B, NFLAT).astype(np.int64)
    kp_vis = (keypoints[..., 1] > 0).reshape(B, NFLAT)
    upper = np.triu(np.ones((NPERS, NPERS), dtype=bool), 1)
    s_all = np.arange(128 * KPJ)
    s_part = s_all // KPJ
    s_col = s_all % KPJ
    s_person = np.minimum(s_all // NKP, NPERS - 1)
    s_valid = s_all < NFLAT

    in_maps = []
    for c in range(NCORES):
        tags_flat = np.ascontiguousarray(
            tags[c * IMGS:(c + 1) * IMGS].reshape(IMGS * N, D), dtype=np.float32
        )
        idx = np.zeros((128, IMGS * KPJ), dtype=np.int32)
        mvw = np.zeros((128, IMGS * W + IMGS * KPJ * D), dtype=np.float32)
        cst2 = np.zeros((30, 488), dtype=np.float32)
        for lb in range(IMGS):
            gb = c * IMGS + lb
            fidx = kp_idx[gb]            # [510]
            fvis = kp_vis[gb]            # [510]
            rows = np.zeros(128 * KPJ, dtype=np.int64)
            rows[:NFLAT] = fidx + lb * N
            vis = np.zeros(128 * KPJ, dtype=np.float32)
            vis[:NFLAT] = fvis.astype(np.float32)
            idx[s_part, lb * KPJ + s_col] = rows
            vis_pk = fvis.reshape(NPERS, NKP)
            cnt = vis_pk.sum(axis=1).astype(np.float32)
            valid = cnt > 0
            safe_cnt = np.maximum(cnt, 1.0)
            invcv = valid / safe_cnt
            mv = vis * s_valid
            mvw[s_part, lb * W + s_col * NPERS + s_person] = mv
            w2v = mv * invcv[s_person]
            for d in range(D):
                mvw[s_part, IMGS * W + lb * KPJ * D + s_col * D + d] = w2v
            cst2[0:D, W + lb * NPERS:W + (lb + 1) * NPERS] = (1.0 / safe_cnt)[None, :]
            cst2[0:D, 2 * W + lb * NPERS:2 * W + (lb + 1) * NPERS] = (
                -2.0 / safe_cnt
            )[None, :]
            cst2[0, 3 * W + lb * NPERS:3 * W + (lb + 1) * NPERS] = 1.0 / (
                safe_cnt * safe_cnt
            )
            cst2[0:NPERS, lb * NPERS:(lb + 1) * NPERS] = (
                upper & valid[:, None] & valid[None, :]
            ).astype(np.float32)
            n = valid.sum().astype(np.float32)
            cst2[0, 4 * W + lb] = 1.0 / ((n - 1.0) * n + EPS)
            cst2[0, 4 * W + IMGS + lb] = 1.0 / (n + EPS)
        in_maps.append({"tags": tags_flat, "idx": idx, "mvw": mvw, "cst2": cst2})
    return in_maps


def kernel(tags: np.ndarray, keypoints: np.ndarray) -> np.ndarray:
    tags = np.asarray(tags, dtype=np.float32)
    keypoints = np.asarray(keypoints)
    nc = _get_nc()
    in_maps = _host_prep(tags, keypoints)
    res = run_bass_kernel_spmd(nc, in_maps, core_ids=list(range(NCORES)))
    outs = [np.asarray(r["out"]).reshape(IMGS, 2) for r in res.results]
    return np.concatenate(outs, axis=0)
